# revision 4
# baseline (speedup 1.0000x reference)
"""Block-local self-attention (BLOCK=128, 3-block windows + global token) on 8
Trainium2 NeuronCores.

Sharding: batch*heads = 32 (n,h) pairs -> 4 pairs per core, no cross-core comms.

Per-core device kernel, per pair:
  - scoresT slabs: for each k-block j (32 of them), one matmul computes
    scoresT[k in block j, q in blocks qlo..qlo+2] + a q0 column, with the
    additive mask folded in as a 65th contraction row (K-side row = mask,
    Q-side row = 1.0) and the 1/sqrt(d) scale folded into Q on the host.
  - exp on ScalarE (batched 2 slabs/op, PSUM->SBUF bf16).
  - PV: ctx[q,d] accumulated in PSUM over the 3 contributing slabs with the
    exp tile as the stationary operand; a 65th V column of ones accumulates
    the softmax denominator in the same matmuls.
  - global slot: every window also attends to token 0's K/V.  e0[q] =
    exp(q.k0*scale + m0) is computed as 32 tiny matmuls into a [128,32]
    PSUM column tile, exp'd, flattened to row layout by an SBUF->SBUF DMA,
    and added to each window as a K=1 rank-1 matmul (V'[0] row).
  - global query row: each slab's q0 column is exp'd with the rest of the
    slab; 32 accumulating [1,65] matmuls against V' give softmax(q0.K) @ V.
  - normalize: DVE reciprocal of the denominator column + tensor_scalar mul.

Output is written in a (pair, mgroup, partition, window, d) layout so every
DMA descriptor row is >= 1KB; the host inverts the layout.
"""

import numpy as np
import ml_dtypes

N, H, T, D = 2, 16, 4000, 64
BLOCK = 128
TP = 4096            # padded token count (32 blocks)
W = 32               # number of 128-blocks
NCORES = 8
PAIRS = N * H        # 32
PPC = PAIRS // NCORES  # pairs per core
SLABW = 3 * BLOCK + 1  # 385: 3 q-blocks + q0 column
NEG = -30000.0
SCALE = 1.0 / np.sqrt(np.float32(D))

_prog_cache = {}


def _qlo(j):
    return min(max(j - 1, 0), W - 3)


def _build_program():
    if "nc" in _prog_cache:
        return _prog_cache["nc"]

    import concourse.bacc as bacc
    import concourse.mybir as mybir
    from concourse import tile

    dt = mybir.dt
    EXP = mybir.ActivationFunctionType.Exp

    nc = bacc.Bacc("TRN2", target_bir_lowering=False, debug=False,
                   num_devices=NCORES)
    qts_d = nc.dram_tensor("qts", [PPC, 65, W * SLABW], dt.bfloat16,
                           kind="ExternalInput").ap()
    kte_d = nc.dram_tensor("kte", [PPC, 65, TP], dt.bfloat16,
                           kind="ExternalInput").ap()
    vp_d = nc.dram_tensor("vp", [PPC, 128, W * 65], dt.bfloat16,
                          kind="ExternalInput").ap()
    out_d = nc.dram_tensor("out", [PPC, 8, 128, 256], dt.float32,
                           kind="ExternalOutput").ap()

    with tile.TileContext(nc) as tc:
        with (
            tc.tile_pool(name="qts", bufs=2) as qts_pool,
            tc.tile_pool(name="kte", bufs=2) as kte_pool,
            tc.tile_pool(name="vp", bufs=2) as vp_pool,
            tc.tile_pool(name="ex", bufs=4) as ex_pool,
            tc.tile_pool(name="small", bufs=3) as small_pool,
            tc.tile_pool(name="outp", bufs=3) as out_pool,
            tc.tile_pool(name="sc", bufs=2, space="PSUM") as sc_pool,
            tc.tile_pool(name="ctx", bufs=3, space="PSUM") as ctx_pool,
            tc.tile_pool(name="aux", bufs=1, space="PSUM") as aux_pool,
            tc.tile_pool(name="dram", bufs=2, space="DRAM") as dram_pool,
        ):
            for p in range(PPC):
                qts_t = qts_pool.tile([65, W * SLABW], dt.bfloat16, tag="qts")
                nc.sync.dma_start(qts_t[:], qts_d[p])
                kte_t = kte_pool.tile([65, TP], dt.bfloat16, tag="kte")
                nc.sync.dma_start(kte_t[:], kte_d[p])
                vp_t = vp_pool.tile([128, W * 65], dt.bfloat16, tag="vp")
                nc.sync.dma_start(vp_t[:], vp_d[p])

                def qblock(i, qts_t=qts_t):
                    # QT block i as a [65, 128] slice of the slab-packed tile
                    if i <= W - 3:
                        s, g = i + 1, i - _qlo(i + 1)
                    else:
                        s, g = W - 1, i - _qlo(W - 1)
                    base = s * SLABW + g * 128
                    return qts_t[:, base:base + 128]

                # ---- e0: token-0 key/value slot scores for every q --------
                s0_ps = aux_pool.tile([128, W], dt.float32, tag="aux")
                for i in range(W):
                    nc.tensor.matmul(s0_ps[:, i:i + 1], qblock(i),
                                     kte_t[:, 0:1], start=True, stop=True)
                e0_sb = small_pool.tile([128, W], dt.bfloat16, tag="e0")
                nc.scalar.activation(e0_sb[:], s0_ps[:], EXP)
                # flatten [128, 32] column layout -> [1, 4096] row layout,
                # bounced through DRAM (the DMA AP balancer cannot fold
                # partitions into the free dim directly)
                e0_dr = dram_pool.tile([W, 128], dt.bfloat16, tag="e0d")
                nc.sync.dma_start(
                    e0_dr[:].rearrange("i p -> p i"), e0_sb[:])
                e0rows = small_pool.tile([1, TP], dt.bfloat16, tag="e0r")
                nc.sync.dma_start(
                    e0rows[:], e0_dr[:].rearrange("i p -> (i p)")[None, :])

                gctx_ps = aux_pool.tile([1, 65], dt.float32, tag="aux")

                ex_tiles = {}
                out_tiles = {}

                def emit_window(w, p=p, vp_t=vp_t, e0rows=e0rows,
                                ex_tiles=ex_tiles, out_tiles=out_tiles):
                    ct = ctx_pool.tile([128, 65], dt.float32, tag="ctx")
                    slabs = [s for s in (w - 1, w, w + 1) if 0 <= s < W]
                    for idx, s in enumerate(slabs):
                        g = w - _qlo(s)
                        exm = ex_tiles[s // 2]
                        base = (s % 2) * SLABW + g * 128
                        nc.tensor.matmul(ct[:], exm[:, base:base + 128],
                                         vp_t[:, s * 65:(s + 1) * 65],
                                         start=(idx == 0), stop=False)
                    # global slot: += e0[q] (x) V'[token 0]
                    nc.tensor.matmul(ct[:], e0rows[0:1, w * 128:(w + 1) * 128],
                                     vp_t[0:1, 0:65], start=False, stop=True)
                    rc = small_pool.tile([128, 1], dt.float32, tag="rc")
                    nc.vector.reciprocal(rc[:], ct[:, 64:65])
                    mi, wi = w // 4, w % 4
                    if wi == 0:
                        out_tiles[mi] = out_pool.tile([128, 256], dt.float32,
                                                      tag="out",
                                                      name=f"out_{p}_{mi}")
                    ot = out_tiles[mi]
                    nc.vector.tensor_scalar_mul(ot[:, wi * 64:(wi + 1) * 64],
                                                ct[:, 0:64], rc[:])
                    if wi == 3:
                        nc.sync.dma_start(out_d[p, mi], ot[:])

                for m in range(W // 2):
                    sc = sc_pool.tile([128, 1024], dt.float32, tag="sc")
                    for h2 in range(2):
                        j = 2 * m + h2
                        nc.tensor.matmul(
                            sc[:, h2 * 512:h2 * 512 + SLABW],
                            kte_t[:, j * 128:(j + 1) * 128],
                            qts_t[:, j * SLABW:(j + 1) * SLABW],
                            start=True, stop=True)
                    ex = ex_pool.tile([128, 2 * SLABW], dt.bfloat16, tag="ex")
                    nc.scalar.activation(
                        ex[:].rearrange("p (b x) -> p b x", x=SLABW),
                        sc[:].rearrange("p (b x) -> p b x", x=512)[:, :, 0:SLABW],
                        EXP)
                    ex_tiles[m] = ex
                    if m == 0:
                        # token 0 is served by the global slot; zero its
                        # window-path row (q0 column kept for the global row)
                        nc.gpsimd.memset(ex[0:1, 0:3 * BLOCK], 0.0)
                    for h2 in range(2):
                        j = 2 * m + h2
                        nc.tensor.matmul(
                            gctx_ps[:],
                            ex[:, h2 * SLABW + 384:h2 * SLABW + 385],
                            vp_t[:, j * 65:(j + 1) * 65],
                            start=(j == 0), stop=(j == W - 1))
                    if m > 0:
                        emit_window(2 * m - 1)
                    emit_window(2 * m)
                    if m == W // 2 - 1:
                        emit_window(W - 1)

                # global query row -> overwrites token 0's output
                rg = small_pool.tile([1, 1], dt.float32, tag="rg")
                nc.vector.reciprocal(rg[:], gctx_ps[0:1, 64:65])
                go = small_pool.tile([1, 64], dt.float32, tag="go")
                nc.vector.tensor_scalar_mul(go[:], gctx_ps[0:1, 0:64], rg[:])
                nc.sync.dma_start(out_d[p, 0, 0:1, 0:64], go[:])

    nc.compile()
    _prog_cache["nc"] = nc
    return nc


def _prep_core_inputs(q, k, v, mask):
    """q,k,v: (PAIRS, T, D) f32; mask: (N, T) f32.  Returns list of per-core
    input dicts (bf16 device layouts)."""
    bf16 = ml_dtypes.bfloat16
    in_maps = []
    for c in range(NCORES):
        qts = np.zeros((PPC, 65, W * SLABW), np.float32)
        kte = np.zeros((PPC, 65, TP), np.float32)
        vp = np.zeros((PPC, 128, W * 65), np.float32)
        for pp in range(PPC):
            pair = c * PPC + pp
            n = pair // H
            m_n = mask[n]
            # QT_ext: [65, TP], rows 0..63 = scale * Q^T, row 64 = 1.0
            QT = np.zeros((65, TP), np.float32)
            QT[:D, :T] = q[pair].T * SCALE
            QT[D, :] = 1.0
            # KT_ext: rows 0..63 = K^T, row 64 = additive mask vector
            KT = np.zeros((65, TP), np.float32)
            KT[:D, :T] = k[pair].T
            KT[D, :T] = m_n
            KT[D, T:] = NEG
            KT[D, 0] = m_n[0]  # token 0 served via the global slot
            kte[pp] = KT
            for j in range(W):
                lo = _qlo(j)
                qts[pp, :, j * SLABW:j * SLABW + 3 * BLOCK] = \
                    QT[:, lo * 128:(lo + 3) * 128]
                qts[pp, :, j * SLABW + 3 * BLOCK] = QT[:, 0]
            # V': (TP, 65) = [V | ones] -> (128, W, 65)
            Vp = np.zeros((TP, 65), np.float32)
            Vp[:T, :D] = v[pair]
            Vp[:, D] = 1.0
            Vp[T:, D] = 1.0  # pad rows get exp=0 anyway; keep denom harmless
            vp[pp] = Vp.reshape(W, 128, 65).transpose(1, 0, 2).reshape(128, W * 65)
        in_maps.append({
            "qts": qts.astype(bf16),
            "kte": kte.astype(bf16),
            "vp": vp.astype(bf16),
        })
    return in_maps


def _unshard(results):
    out = np.empty((PAIRS, T, D), np.float32)
    for c in range(NCORES):
        o = results[c]["out"]  # (PPC, 8, 128, 256)
        o = o.reshape(PPC, 8, 128, 4, 64).transpose(0, 1, 3, 2, 4)
        o = o.reshape(PPC, TP, D)[:, :T, :]
        out[c * PPC:(c + 1) * PPC] = o
    return out.reshape(N, H, T, D)


def _run(inputs, trace=False, tmpdir=None):
    from concourse.bass_utils import run_bass_kernel_spmd

    q = np.asarray(inputs["query_layer"], np.float32).reshape(PAIRS, T, D)
    k = np.asarray(inputs["key_layer"], np.float32).reshape(PAIRS, T, D)
    v = np.asarray(inputs["value_layer"], np.float32).reshape(PAIRS, T, D)
    mask = np.asarray(inputs["attention_mask"], np.float32).reshape(N, T)

    nc = _build_program()
    in_maps = _prep_core_inputs(q, k, v, mask)
    res = run_bass_kernel_spmd(nc, in_maps, list(range(NCORES)),
                               trace=trace, tmpdir=tmpdir)
    return _unshard(res.results), res


def kernel(query_layer, key_layer, value_layer, attention_mask):
    out, _ = _run({
        "query_layer": query_layer,
        "key_layer": key_layer,
        "value_layer": value_layer,
        "attention_mask": attention_mask,
    })
    return out
